# revision 24
# baseline (speedup 1.0000x reference)
"""DIVeR forward kernel for Trainium2 (8 NeuronCores, data-parallel over rays).

Self-contained: builds a Bass/Tile kernel, shards B=1024 rays across 8 cores,
runs via PJRT (axon), and post-processes on host (reorder + final pointwise
activations + masking, ~0.1% of total FLOPs).
"""
import functools
import numpy as np

import concourse.bass as bass
import concourse.bacc as bacc
import concourse.tile as tile
import concourse.mybir as mybir

f32 = mybir.dt.float32
f32r = mybir.dt.float32r
i32 = mybir.dt.int32
u8 = mybir.dt.uint8
AT = mybir.ActivationFunctionType
OP = mybir.AluOpType
AX = mybir.AxisListType

# problem constants
VOXEL_NUM = 128
VOXEL_DIM = 32
GRID_SIZE = 2.0
VOXEL_SIZE = GRID_SIZE / VOXEL_NUM
XYZMAX = 1.0
XYZMIN = -1.0
DIR_ENCODE = 4
B = 1024
K = 3 * (VOXEL_NUM + 1) + 2      # 389 candidates
M = K - 1                        # 388 segments
BIG = 1e10
NCORES = 8
RAYS = B // NCORES               # 128 rays per core
NV = VOXEL_NUM + 1               # 129
NBLK = 4                         # m blocks of 128 (pad M 388->512)
MP = 512                         # padded M
NQUAD = RAYS // 4                # 32 ray-quads -> 128 tiles of 512 slots
NTILES = NBLK * NQUAD


def _ap(t_ap, extra_dims, offset=0):
    """Build an AP with explicit free dims on top of a tile's partition dim."""
    return bass.AP(t_ap.tensor, t_ap.offset + offset,
                   [list(t_ap.ap[0])] + [list(d) for d in extra_dims])


def build_module(repeat=1, nblk_active=3):
    nc = bacc.Bacc("TRN2", target_bir_lowering=False, debug=False,
                   num_devices=NCORES)

    dram = {}
    def din(name, shape, dtype=f32):
        dram[name] = nc.dram_tensor(name, shape, dtype, kind="ExternalInput").ap()
    def dout(name, shape, dtype=f32):
        dram[name] = nc.dram_tensor(name, shape, dtype, kind="ExternalOutput").ap()

    din("osd", [RAYS, 3]); din("dsd", [RAYS, 3])
    din("vox", [NV * NV * NV, VOXEL_DIM])
    din("planes", [RAYS, NV])           # replicated plane coords
    din("ident", [128, 128])            # PE transpose identity
    din("w0", [32, 256]); din("b0", [128, 2])
    din("w1", [256, 256]); din("b1", [128, 2])
    din("w2", [256, 66]); din("b2", [128, 1])
    din("v0h", [66, 256])               # [zeros(2); m2w0 rows 0:64]
    din("v1", [256, 256]); din("vb1", [128, 2])
    din("v2", [256, 3]); din("vb2", [128, 1])
    din("vct", [RAYS, 256])             # venc @ m2w0[64:] + m2b0, per ray
    din("onehot", [4, 512])             # row r: cols 128*r.. = 1

    import os as _os
    if _os.environ.get("KDEBUG", "0") == "1":
        dout("d_ta", [RAYS, 512]); dout("d_sorted", [RAYS, 512])
        dout("d_inv", [RAYS, 3]); dout("d_bc", [RAYS, 8])
    dout("o_x01", [NTILES, 2, 512])     # pre-softplus sigma/beta, tile-major
    dout("o_color", [NTILES, 3, 512])   # pre-sigmoid color
    dout("o_seg", [RAYS, M], u8)
    dout("o_tsout", [RAYS, M])

    with tile.TileContext(nc) as tc:
        def body():
            kernel_body(nc, tc, dram, nblk_active)
        if repeat == 1:
            body()
        else:
            with tc.For_i(0, repeat, 1):
                body()
    nc.compile()
    return nc


def kernel_body(nc, tc, dram, nblk_active=3):
    from contextlib import ExitStack
    with ExitStack() as ctx:
        cpool = ctx.enter_context(tc.tile_pool(name="consts", bufs=1))
        ppool = ctx.enter_context(tc.tile_pool(name="persist", bufs=1))
        bpool = ctx.enter_context(tc.tile_pool(name="blk", bufs=2))
        mpool = ctx.enter_context(tc.tile_pool(name="main", bufs=2))
        psA = ctx.enter_context(tc.tile_pool(name="psA", bufs=1, space="PSUM"))
        psB = ctx.enter_context(tc.tile_pool(name="psB", bufs=4, space="PSUM"))
        psS = ctx.enter_context(tc.tile_pool(name="psS", bufs=1, space="PSUM"))

        V = nc.vector
        S = nc.scalar
        G = nc.gpsimd

        # ---- load constants ----
        def cload(name, shape, dtype=f32, round_r=False):
            t = cpool.tile(shape, dtype, name=name + "_c")
            nc.sync.dma_start(t[:], dram[name][:])
            if round_r:
                tr = cpool.tile(shape, f32r, name=name + "_r")
                V.tensor_copy(tr[:], t[:])
                return tr
            return t

        ident = cload("ident", [128, 128])
        w0 = cload("w0", [32, 256], round_r=True)
        # w1/w2/v1/v2 need K-chunk tiles of <=128 partitions
        def cload2(name, kdim, mdim):
            ts = []
            for kk in range((kdim + 127) // 128):
                p = min(128, kdim - kk * 128)
                t = cpool.tile([p, mdim], f32, name=f"{name}_{kk}_c")
                nc.sync.dma_start(t[:], dram[name][kk * 128:kk * 128 + p, :])
                tr = cpool.tile([p, mdim], f32r, name=f"{name}_{kk}_r")
                V.tensor_copy(tr[:], t[:])
                ts.append(tr)
            return ts
        w1c = cload2("w1", 256, 256)
        w2c = cload2("w2", 256, 66)
        v0h = cload2("v0h", 66, 256)[0]
        v1c = cload2("v1", 256, 256)
        v2c = cload2("v2", 256, 3)
        b0 = cload("b0", [128, 2]); b1 = cload("b1", [128, 2]); b2 = cload("b2", [128, 1])
        vb1 = cload("vb1", [128, 2]); vb2 = cload("vb2", [128, 1])
        vct = cload("vct", [RAYS, 256], round_r=True)
        onehot = cload("onehot", [4, 512], round_r=True)
        planes = cload("planes", [RAYS, NV])

        bigt = cpool.tile([128, 512], f32, name="bigt")
        G.memset(bigt[:], BIG)
        epst = cpool.tile([128, 4], f32, name="epst")
        G.memset(epst[:], 1e-9)
        zero512 = cpool.tile([128, 512], f32, name="zero512")
        G.memset(zero512[:], 0.0)

        # ---- stage A ----
        astack = ExitStack()
        apool = astack.enter_context(tc.tile_pool(name="stagea", bufs=1))
        osd = apool.tile([RAYS, 3], f32, name="osd")
        nc.sync.dma_start(osd[:], dram["osd"][:])
        dsd = apool.tile([RAYS, 3], f32, name="dsd")
        nc.sync.dma_start(dsd[:], dram["dsd"][:])

        negd = apool.tile([RAYS, 3], f32, name="negd")
        V.tensor_scalar(out=negd[:], in0=dsd[:], scalar1=-1.0, scalar2=None, op0=OP.mult)
        absd = apool.tile([RAYS, 3], f32, name="absd")
        V.tensor_tensor(out=absd[:], in0=dsd[:], in1=negd[:], op=OP.max)
        small = apool.tile([RAYS, 3], u8, name="small")
        V.tensor_scalar(out=small[:], in0=absd[:], scalar1=1e-9, scalar2=None, op0=OP.is_lt)
        dsafe = apool.tile([RAYS, 3], f32, name="dsafe")
        V.select(dsafe[:], small[:], epst[:, :3], dsd[:])
        inv = apool.tile([RAYS, 3], f32, name="inv")
        V.reciprocal(inv[:], dsafe[:])

        # candidates T: [0]=tmin [1]=tmax [2:389]=tc, [389:]=BIG
        Ta = apool.tile([RAYS, 512], f32, name="Ta")
        Tb = apool.tile([RAYS, 512], f32, name="Tb")
        G.memset(Ta[:, 389:], BIG)
        G.memset(Tb[:, 389:], BIG)
        for a in range(3):
            V.tensor_scalar(out=Ta[:, 2 + NV * a:2 + NV * (a + 1)], in0=planes[:],
                            scalar1=osd[:, a:a + 1], scalar2=inv[:, a:a + 1],
                            op0=OP.subtract, op1=OP.mult)
        # tmin/tmax from boundary cols
        bc = apool.tile([RAYS, 8], f32, name="bc")
        for a in range(3):
            V.tensor_tensor(out=bc[:, a:a + 1], in0=Ta[:, 2 + NV * a:3 + NV * a],
                            in1=Ta[:, 2 + NV * a + 128:3 + NV * a + 128], op=OP.min)
            V.tensor_tensor(out=bc[:, 4 + a:5 + a], in0=Ta[:, 2 + NV * a:3 + NV * a],
                            in1=Ta[:, 2 + NV * a + 128:3 + NV * a + 128], op=OP.max)
        V.tensor_tensor(out=bc[:, 0:1], in0=bc[:, 0:1], in1=bc[:, 1:2], op=OP.max)
        V.tensor_scalar(out=bc[:, 0:1], in0=bc[:, 0:1], scalar1=bc[:, 2:3], scalar2=0.0,
                        op0=OP.max, op1=OP.max)  # tmin
        V.tensor_tensor(out=bc[:, 4:5], in0=bc[:, 4:5], in1=bc[:, 5:6], op=OP.min)
        V.tensor_tensor(out=bc[:, 4:5], in0=bc[:, 4:5], in1=bc[:, 6:7], op=OP.min)  # tmax
        tmin = bc[:, 0:1]; tmax = bc[:, 4:5]
        hitf = apool.tile([RAYS, 1], f32, name="hitf")
        V.tensor_tensor(out=hitf[:], in0=tmax, in1=tmin, op=OP.is_gt)
        V.tensor_copy(Ta[:, 0:1], tmin)
        V.tensor_copy(Ta[:, 1:2], tmax)
        # validity
        lo = apool.tile([RAYS, 1], f32, name="lo")
        V.tensor_scalar(out=lo[:], in0=tmin, scalar1=-1e-6, scalar2=None, op0=OP.add)
        hi = apool.tile([RAYS, 1], f32, name="hi")
        V.tensor_scalar(out=hi[:], in0=tmax, scalar1=1e-6, scalar2=None, op0=OP.add)
        vmf = apool.tile([RAYS, 512], f32, name="vmf")
        V.tensor_scalar(out=vmf[:, :389], in0=Ta[:, :389], scalar1=lo[:], scalar2=None, op0=OP.is_ge)
        vm2 = apool.tile([RAYS, 512], f32, name="vm2")
        V.tensor_scalar(out=vm2[:, :389], in0=Ta[:, :389], scalar1=hi[:], scalar2=None, op0=OP.is_le)
        V.tensor_tensor(out=vmf[:, :389], in0=vmf[:, :389], in1=vm2[:, :389], op=OP.mult)
        V.tensor_scalar(out=vmf[:, :389], in0=vmf[:, :389], scalar1=hitf[:], scalar2=None,
                        op0=OP.mult)
        vmask = apool.tile([RAYS, 512], u8, name="vmask")
        V.tensor_copy(vmask[:, :389], vmf[:, :389])
        V.select(Tb[:, :389], vmask[:, :389], Ta[:, :389], bigt[:, :389])

        import os as _os
        if _os.environ.get("KDEBUG", "0") == "1":
            nc.sync.dma_start(dram["d_ta"][:], Ta[:])
            nc.sync.dma_start(dram["d_inv"][:], inv[:])
            nc.sync.dma_start(dram["d_bc"][:], bc[:])
        # ---- bitonic sort (ascending, 512) ----
        cur, nxt = Tb, Ta
        for p in range(1, 10):
            s = 1 << p
            nb = 512 // s
            h = s // 2
            A = _ap(cur[:], [[s, nb], [1, h]])
            Br = _ap(cur[:], [[s, nb], [-1, h]], offset=s - 1)
            Ao = _ap(nxt[:], [[s, nb], [1, h]])
            Bo = _ap(nxt[:], [[s, nb], [-1, h]], offset=s - 1)
            V.tensor_tensor(out=Ao, in0=A, in1=Br, op=OP.min)
            V.tensor_tensor(out=Bo, in0=A, in1=Br, op=OP.max)
            cur, nxt = nxt, cur
            d = s // 4
            while d >= 1:
                nb2 = 512 // (2 * d)
                A = _ap(cur[:], [[2 * d, nb2], [1, d]])
                Bv = _ap(cur[:], [[2 * d, nb2], [1, d]], offset=d)
                Ao = _ap(nxt[:], [[2 * d, nb2], [1, d]])
                Bo = _ap(nxt[:], [[2 * d, nb2], [1, d]], offset=d)
                V.tensor_tensor(out=Ao, in0=A, in1=Bv, op=OP.min)
                V.tensor_tensor(out=Bo, in0=A, in1=Bv, op=OP.max)
                cur, nxt = nxt, cur
                d //= 2
        ts_s = cur  # sorted; [:, :389] are the K candidates
        if _os.environ.get("KDEBUG", "0") == "1":
            nc.sync.dma_start(dram["d_sorted"][:], ts_s[:])

        # ---- masks & outputs ts/seg ----
        pmu = apool.tile([RAYS, 389], u8, name="pmu")
        V.tensor_scalar(out=pmu[:], in0=ts_s[:, :389], scalar1=0.5 * BIG, scalar2=None, op0=OP.is_lt)
        pmf = apool.tile([RAYS, 389], f32, name="pmf")
        V.tensor_scalar(out=pmf[:], in0=ts_s[:, :389], scalar1=0.5 * BIG, scalar2=None, op0=OP.is_lt)
        tso = apool.tile([RAYS, M], f32, name="tso")
        V.tensor_tensor(out=tso[:], in0=ts_s[:, :M], in1=pmf[:, :M], op=OP.mult)
        nc.sync.dma_start(dram["o_tsout"][:], tso[:])
        dtt = apool.tile([RAYS, M], f32, name="dtt")
        V.tensor_tensor(out=dtt[:], in0=ts_s[:, 1:389], in1=ts_s[:, :M], op=OP.subtract)
        segu = apool.tile([RAYS, M], u8, name="segu")
        V.tensor_scalar(out=segu[:], in0=dtt[:], scalar1=1e-6, scalar2=None, op0=OP.is_gt)
        V.tensor_tensor(out=segu[:], in0=segu[:], in1=pmu[:, :M], op=OP.logical_and)
        V.tensor_tensor(out=segu[:], in0=segu[:], in1=pmu[:, 1:389], op=OP.logical_and)
        nc.sync.dma_start(dram["o_seg"][:], segu[:])
        segf = apool.tile([RAYS, M], f32, name="segf")
        V.tensor_copy(segf[:], segu[:])

        # ---- coords / idx / frac ----
        coord = [apool.tile([RAYS, 389], f32, name=f"coord{a}") for a in range(3)]
        for a in range(3):
            V.tensor_scalar(out=coord[a][:], in0=ts_s[:, :389], scalar1=dsd[:, a:a + 1],
                            scalar2=osd[:, a:a + 1], op0=OP.mult, op1=OP.add)
            V.tensor_scalar(out=coord[a][:], in0=coord[a][:], scalar1=1.0, scalar2=64.0,
                            op0=OP.add, op1=OP.mult)
            V.tensor_scalar(out=coord[a][:], in0=coord[a][:], scalar1=0.0, scalar2=128.0,
                            op0=OP.max, op1=OP.min)
        idxf = []
        for a in range(3):
            mid = apool.tile([RAYS, M], f32, name="mid")
            V.tensor_tensor(out=mid[:], in0=coord[a][:, :M], in1=coord[a][:, 1:389], op=OP.add)
            V.tensor_scalar(out=mid[:], in0=mid[:], scalar1=0.5, scalar2=None, op0=OP.mult)
            ri = apool.tile([RAYS, M], i32, name="ri")
            V.tensor_copy(ri[:], mid[:])              # round-to-nearest on hw
            rf = apool.tile([RAYS, M], f32, name=f"rf{a}")
            V.tensor_copy(rf[:], ri[:])
            gt = apool.tile([RAYS, M], f32, name="gt")
            V.tensor_tensor(out=gt[:], in0=rf[:], in1=mid[:], op=OP.is_gt)
            V.tensor_tensor(out=rf[:], in0=rf[:], in1=gt[:], op=OP.subtract)  # floor
            V.tensor_scalar(out=rf[:], in0=rf[:], scalar1=0.0, scalar2=127.0,
                            op0=OP.max, op1=OP.min)
            idxf.append(rf)

        # frac arrays a/b and midpoint m per axis
        pa = []; pb = []; pm = []
        for a in range(3):
            t1 = apool.tile([RAYS, M], f32, name=f"pa{a}")
            V.tensor_tensor(out=t1[:], in0=coord[a][:, :M], in1=idxf[a][:], op=OP.subtract)
            t2 = apool.tile([RAYS, M], f32, name=f"pb{a}")
            V.tensor_tensor(out=t2[:], in0=coord[a][:, 1:389], in1=idxf[a][:], op=OP.subtract)
            t3 = apool.tile([RAYS, M], f32, name=f"pm{a}")
            V.tensor_tensor(out=t3[:], in0=t1[:], in1=t2[:], op=OP.add)
            V.tensor_scalar(out=t3[:], in0=t3[:], scalar1=0.5, scalar2=None, op0=OP.mult)
            pa.append(t1); pb.append(t2); pm.append(t3)

        # length factor lf = sqrt(sum dd^2) * VOXEL_SIZE/6 * segmask
        lf = apool.tile([RAYS, M], f32, name="lf")
        acc = apool.tile([RAYS, M], f32, name="lacc")
        for a in range(3):
            dd = apool.tile([RAYS, M], f32, name="ldd")
            V.tensor_tensor(out=dd[:], in0=coord[a][:, 1:389], in1=coord[a][:, :M], op=OP.subtract)
            if a == 0:
                V.tensor_tensor(out=acc[:], in0=dd[:], in1=dd[:], op=OP.mult)
            else:
                sq = apool.tile([RAYS, M], f32, name="lsq")
                V.tensor_tensor(out=sq[:], in0=dd[:], in1=dd[:], op=OP.mult)
                V.tensor_tensor(out=acc[:], in0=acc[:], in1=sq[:], op=OP.add)
        S.activation(lf[:], acc[:], AT.Sqrt)
        V.tensor_scalar(out=lf[:], in0=lf[:], scalar1=VOXEL_SIZE / 6.0, scalar2=None, op0=OP.mult)
        V.tensor_tensor(out=lf[:], in0=lf[:], in1=segf[:], op=OP.mult)

        # corner weights wf[k] = (wa_k + 4*wm_k + wb_k) * lf ; k = dx*4+dy*2+dz
        wf = ppool.tile([RAYS, 8, 512], f32, name="wf")
        G.memset(wf[:], 0.0)
        wacc = [apool.tile([RAYS, M], f32, name=f"wacc{kk}") for kk in range(8)]
        tmpw = apool.tile([RAYS, M], f32, name="tmpw")
        for e, P in enumerate((pm, pa, pb)):   # m first: wacc = 4*wm, then += wa, wb
            onem = []
            for a in range(3):
                t = apool.tile([RAYS, M], f32, name=f"om{a}")
                V.tensor_scalar(out=t[:], in0=P[a][:], scalar1=-1.0, scalar2=1.0,
                                op0=OP.mult, op1=OP.add)
                onem.append(t)
            xy = {}
            for dx in range(2):
                for dy in range(2):
                    t = apool.tile([RAYS, M], f32, name=f"xy{dx}{dy}")
                    fx = P[0] if dx else onem[0]
                    fy = P[1] if dy else onem[1]
                    V.tensor_tensor(out=t[:], in0=fx[:], in1=fy[:], op=OP.mult)
                    xy[(dx, dy)] = t
            for kk in range(8):
                dx, dy, dz = kk >> 2, (kk >> 1) & 1, kk & 1
                fz = P[2] if dz else onem[2]
                if e == 0:
                    V.tensor_tensor(out=tmpw[:], in0=xy[(dx, dy)][:], in1=fz[:], op=OP.mult)
                    V.tensor_scalar(out=wacc[kk][:], in0=tmpw[:], scalar1=4.0, scalar2=None,
                                    op0=OP.mult)
                else:
                    V.tensor_tensor(out=tmpw[:], in0=xy[(dx, dy)][:], in1=fz[:], op=OP.mult)
                    V.tensor_tensor(out=wacc[kk][:], in0=wacc[kk][:], in1=tmpw[:], op=OP.add)
        for kk in range(8):
            V.tensor_tensor(out=wf[:, kk, :M], in0=wacc[kk][:], in1=lf[:], op=OP.mult)

        # gather index per corner-pair q = dx*2+dy: gf = flat voxel id (f32 exact)
        gf = ppool.tile([RAYS, 4, 512], f32, name="gf")
        G.memset(gf[:], 0.0)
        gb = apool.tile([RAYS, M], f32, name="gb")
        V.scalar_tensor_tensor(out=gb[:], in0=idxf[0][:], scalar=float(NV), in1=idxf[1][:],
                               op0=OP.mult, op1=OP.add)
        V.scalar_tensor_tensor(out=gb[:], in0=gb[:], scalar=float(NV), in1=idxf[2][:],
                               op0=OP.mult, op1=OP.add)
        for q in range(4):
            dx, dy = q >> 1, q & 1
            off = float(dx * NV * NV + dy * NV)
            V.tensor_scalar(out=gf[:, q, :M], in0=gb[:], scalar1=off, scalar2=None, op0=OP.add)

        astack.close()

        # ---- per-block transpose to slot-major, then main loop ----
        for b in range(nblk_active):
            wT = bpool.tile([128, 8, 128], f32, name="wT")
            gT = bpool.tile([128, 128, 4], i32, name="gT")
            for kk in range(8):
                pt = psA.tile([128, 128], f32, name="ptA")
                nc.tensor.transpose(pt[:], wf[:, kk, 128 * b:128 * (b + 1)], ident[:])
                V.tensor_copy(wT[:, kk, :], pt[:])
            for q in range(4):
                pt = psA.tile([128, 128], f32, name="ptA")
                nc.tensor.transpose(pt[:], gf[:, q, 128 * b:128 * (b + 1)], ident[:])
                V.tensor_copy(gT[:, :, q], pt[:])

            for oct_ in range(NQUAD // 2):   # 16 octets of 8 rays
                fc = mpool.tile([128, 8, 4, 64], f32, name="fc")
                idx_ap = _ap(gT[:], [[4, 8], [1, 4]], offset=32 * oct_)
                nc.gpsimd.indirect_dma_start(
                    fc[:].rearrange("p a b c -> p (a b c)"), None, dram["vox"][:, :],
                    bass.IndirectOffsetOnAxis(ap=idx_ap, axis=0))
                for hh in range(2):
                    q4 = 2 * oct_ + hh       # ray-quad index
                    t_idx = b * NQUAD + q4
                    r0 = 4 * q4
                    # product [128, (ray4, c32, k8)]
                    prod = mpool.tile([128, 1024], f32, name="prod")
                    fc_ap = _ap(fc[:], [[256, 4], [1, 32], [32, 8]], offset=1024 * hh)
                    wt_ap = _ap(wT[:], [[1, 4], [0, 32], [128, 8]], offset=r0)
                    pr_ap = _ap(prod[:], [[256, 4], [8, 32], [1, 8]])
                    V.tensor_tensor(out=pr_ap, in0=fc_ap, in1=wt_ap, op=OP.mult)
                    feat = mpool.tile([128, 128], f32, name="feat")
                    V.tensor_reduce(feat[:], prod[:].rearrange("p (g k) -> p g k", k=8),
                                    AX.X, OP.add)
                    # transpose feat -> [32, 512] slots
                    ftp = psS.tile([32, 512], f32, name="ftp")
                    for j in range(4):
                        nc.tensor.transpose(ftp[:, 128 * j:128 * (j + 1)],
                                            feat[:, 32 * j:32 * (j + 1)], ident[:])
                    featT = mpool.tile([32, 512], f32r, name="featT")
                    V.tensor_copy(featT[:], ftp[:])
                    # MLP1 L1
                    h1 = mpool.tile([128, 2, 512], f32r, name="hA")
                    for mh in range(2):
                        p = psB.tile([128, 512], f32, name="pB")
                        nc.tensor.matmul(p[:], w0[:, 128 * mh:128 * (mh + 1)], featT[:],
                                         start=True, stop=True)
                        S.activation(h1[:, mh, :], p[:], AT.Relu, bias=b0[:, mh:mh + 1])
                    # L2
                    h2 = mpool.tile([128, 2, 512], f32r, name="hB")
                    for mh in range(2):
                        p = psB.tile([128, 512], f32, name="pB")
                        nc.tensor.matmul(p[:], w1c[0][:, 128 * mh:128 * (mh + 1)], h1[:, 0, :],
                                         start=True, stop=False)
                        nc.tensor.matmul(p[:], w1c[1][:, 128 * mh:128 * (mh + 1)], h1[:, 1, :],
                                         start=False, stop=True)
                        if mh == 0:
                            S.activation(h2[:, mh, :], p[:], AT.Relu, bias=b1[:, mh:mh + 1])
                        else:
                            V.scalar_tensor_tensor(out=h2[:, mh, :], in0=p[:],
                                                   scalar=b1[:, mh:mh + 1], in1=zero512[:],
                                                   op0=OP.add, op1=OP.max)
                    # L3 -> [66, 512]
                    p3 = psS.tile([66, 512], f32, name="p3")
                    nc.tensor.matmul(p3[:], w2c[0][:], h2[:, 0, :], start=True, stop=False)
                    nc.tensor.matmul(p3[:], w2c[1][:], h2[:, 1, :], start=False, stop=True)
                    x2h = mpool.tile([66, 512], f32r, name="x2h")
                    S.activation(x2h[:], p3[:], AT.Identity, bias=b2[0:66, :])
                    x01 = mpool.tile([2, 512], f32, name="x01")
                    S.activation(x01[:], p3[0:2, :], AT.Identity, bias=b2[0:2, :])
                    nc.sync.dma_start(dram["o_x01"][t_idx, :, :], x01[:])
                    # MLP2 L1: h-part (K=64) + venc contribution (K=4 one-hot)
                    vct4 = mpool.tile([4, 256], f32r, name="vct4")
                    nc.sync.dma_start(vct4[:], vct[r0:r0 + 4, :])
                    g1 = mpool.tile([128, 2, 512], f32r, name="hA")
                    for mh in range(2):
                        p = psB.tile([128, 512], f32, name="pB")
                        nc.tensor.matmul(p[:], v0h[:, 128 * mh:128 * (mh + 1)], x2h[:],
                                         start=True, stop=False)
                        nc.tensor.matmul(p[:], vct4[:, 128 * mh:128 * (mh + 1)],
                                         onehot[:], start=False, stop=True)
                        if mh == 0:
                            S.activation(g1[:, mh, :], p[:], AT.Relu)
                        else:
                            V.scalar_tensor_tensor(out=g1[:, mh, :], in0=p[:],
                                                   scalar=0.0, in1=zero512[:],
                                                   op0=OP.add, op1=OP.max)
                    # MLP2 L2
                    g2 = mpool.tile([128, 2, 512], f32r, name="hB")
                    for mh in range(2):
                        p = psB.tile([128, 512], f32, name="pB")
                        nc.tensor.matmul(p[:], v1c[0][:, 128 * mh:128 * (mh + 1)], g1[:, 0, :],
                                         start=True, stop=False)
                        nc.tensor.matmul(p[:], v1c[1][:, 128 * mh:128 * (mh + 1)], g1[:, 1, :],
                                         start=False, stop=True)
                        if mh == 0:
                            S.activation(g2[:, mh, :], p[:], AT.Relu, bias=vb1[:, mh:mh + 1])
                        else:
                            V.scalar_tensor_tensor(out=g2[:, mh, :], in0=p[:],
                                                   scalar=vb1[:, mh:mh + 1], in1=zero512[:],
                                                   op0=OP.add, op1=OP.max)
                    # MLP2 L3 -> [3, 512]
                    pc = psS.tile([3, 512], f32, name="pc")
                    nc.tensor.matmul(pc[:], v2c[0][:], g2[:, 0, :], start=True, stop=False)
                    nc.tensor.matmul(pc[:], v2c[1][:], g2[:, 1, :], start=False, stop=True)
                    col = mpool.tile([3, 512], f32, name="col")
                    S.activation(col[:], pc[:], AT.Identity, bias=vb2[0:3, :])
                    nc.sync.dma_start(dram["o_color"][t_idx, :, :], col[:])


# ---------------- host side ----------------

def _posenc(d):
    freqs = (2.0 ** np.arange(DIR_ENCODE)).astype(np.float32)
    ang = d[:, None, :].astype(np.float32) * freqs[:, None]
    n = d.shape[0]
    return np.concatenate([d, np.sin(ang).reshape(n, -1), np.cos(ang).reshape(n, -1)],
                          -1).astype(np.float32)


def _host_inputs(inputs):
    os_, ds = np.asarray(inputs["os"]), np.asarray(inputs["ds"])
    vox = np.ascontiguousarray(np.asarray(inputs["voxels"]).reshape(NV * NV * NV, VOXEL_DIM))
    planes = np.broadcast_to(
        (XYZMIN + VOXEL_SIZE * np.arange(NV, dtype=np.float32))[None, :], (RAYS, NV)).copy()
    ident = np.eye(128, dtype=np.float32)
    venc = _posenc(ds)  # (B, 27)
    vct_full = (venc.astype(np.float64) @ np.asarray(inputs["m2w0"])[64:91].astype(np.float64)
                + np.asarray(inputs["m2b0"]).astype(np.float64)).astype(np.float32)  # (B, 256)
    onehot = np.zeros((4, 512), dtype=np.float32)
    for r in range(4):
        onehot[r, 128 * r:128 * (r + 1)] = 1.0

    def pad_bias(b, cols):
        b = np.asarray(b).astype(np.float32)
        out = np.zeros((128, cols), dtype=np.float32)
        for c in range(cols):
            seg = b[128 * c:128 * (c + 1)]
            out[:len(seg), c] = seg
        return out

    common = dict(
        vox=vox, planes=planes, ident=ident, onehot=onehot,
        w0=np.asarray(inputs["m1w0"]).astype(np.float32),
        b0=pad_bias(inputs["m1b0"], 2),
        w1=np.asarray(inputs["m1w1"]).astype(np.float32),
        b1=pad_bias(inputs["m1b1"], 2),
        w2=np.asarray(inputs["m1w2"]).astype(np.float32),
        b2=pad_bias(inputs["m1b2"], 1),
        v0h=np.concatenate([np.zeros((2, 256), np.float32),
                            np.asarray(inputs["m2w0"])[0:64].astype(np.float32)]),
        v1=np.asarray(inputs["m2w1"]).astype(np.float32),
        vb1=pad_bias(inputs["m2b1"], 2),
        v2=np.asarray(inputs["m2w2"]).astype(np.float32),
        vb2=pad_bias(inputs["m2b2"], 1),
    )
    in_maps = []
    for c in range(NCORES):
        sl = slice(RAYS * c, RAYS * (c + 1))
        m = dict(common)
        m["osd"] = os_[sl].astype(np.float32)
        m["dsd"] = ds[sl].astype(np.float32)
        m["vct"] = vct_full[sl]
        in_maps.append(m)
    return in_maps


def _host_post(outs_per_core):
    """outs: list per core of dict with o_x01 [128,2,512], o_color [128,3,512],
    o_seg [128,388] u8, o_tsout [128,388]."""
    color = np.zeros((B, M, 3), np.float32)
    sigma = np.zeros((B, M), np.float32)
    beta = np.zeros((B, M), np.float32)
    segmask = np.zeros((B, M), bool)
    ts_out = np.zeros((B, M), np.float32)
    for c, o in enumerate(outs_per_core):
        sl = slice(RAYS * c, RAYS * (c + 1))
        seg = o["o_seg"].astype(bool)
        segmask[sl] = seg
        ts_out[sl] = o["o_tsout"]
        # tile-major [NTILES(b,q), ch, 512(r,i)] -> [ray, m]
        x01 = o["o_x01"].reshape(NBLK, NQUAD, 2, 4, 128)
        x01 = x01.transpose(1, 3, 2, 0, 4).reshape(RAYS, 2, MP)[:, :, :M]
        col = o["o_color"].reshape(NBLK, NQUAD, 3, 4, 128)
        col = col.transpose(1, 3, 2, 0, 4).reshape(RAYS, 3, MP)[:, :, :M]
        fm = seg.astype(np.float64)
        x01 = np.where(seg[:, None, :], x01, 0.0).astype(np.float64)
        col = np.where(seg[:, None, :], col, 0.0).astype(np.float64)
        sp = np.where(x01 > 30, x01, np.log1p(np.exp(np.minimum(x01, 30.0))))
        sigma[sl] = (sp[:, 0] * fm).astype(np.float32)
        beta[sl] = (sp[:, 1] * fm).astype(np.float32)
        cs = 1.0 / (1.0 + np.exp(-col))
        color[sl] = (cs * fm[:, None]).transpose(0, 2, 1).astype(np.float32)
    return color, sigma, beta, segmask, ts_out


# ---------------- runner ----------------

class _Runner:
    def __init__(self, nc, n_cores):
        import jax
        from jax.sharding import Mesh, PartitionSpec
        from jax.experimental.shard_map import shard_map
        from concourse.bass2jax import _bass_exec_p, install_neuronx_cc_hook, partition_id_tensor
        install_neuronx_cc_hook()
        self.jax = jax
        self.n_cores = n_cores
        partition_name = nc.partition_id_tensor.name if nc.partition_id_tensor else None
        in_names, out_names, out_avals = [], [], []
        for alloc in nc.m.functions[0].allocations:
            if not isinstance(alloc, mybir.MemoryLocationSet):
                continue
            name = alloc.memorylocations[0].name
            if alloc.kind == "ExternalInput":
                if name != partition_name:
                    in_names.append(name)
            elif alloc.kind == "ExternalOutput":
                out_names.append(name)
                out_avals.append(jax.core.ShapedArray(tuple(alloc.tensor_shape),
                                                      mybir.dt.np(alloc.dtype)))
        self.in_names, self.out_names, self.out_avals = in_names, out_names, out_avals
        n_params, n_outs = len(in_names), len(out_avals)
        all_in = list(in_names) + out_names
        if partition_name is not None:
            all_in.append(partition_name)

        def _body(*args):
            operands = list(args)
            if partition_name is not None:
                operands.append(partition_id_tensor())
            return tuple(_bass_exec_p.bind(
                *operands, out_avals=tuple(out_avals), in_names=tuple(all_in),
                out_names=tuple(out_names), lowering_input_output_aliases=(),
                sim_require_finite=False, sim_require_nnan=False, nc=nc))

        devices = jax.devices()[:n_cores]
        self.mesh = Mesh(np.asarray(devices), ("core",))
        self.fn = jax.jit(
            shard_map(_body, mesh=self.mesh,
                      in_specs=(PartitionSpec("core"),) * (n_params + n_outs),
                      out_specs=(PartitionSpec("core"),) * n_outs, check_rep=False),
            keep_unused=True)

    def stage(self, in_maps):
        from jax.sharding import NamedSharding, PartitionSpec
        jax = self.jax
        concat = [np.concatenate([np.asarray(in_maps[c][n]) for c in range(self.n_cores)], axis=0)
                  for n in self.in_names]
        zeros = [np.zeros((self.n_cores * a.shape[0], *a.shape[1:]), a.dtype)
                 for a in self.out_avals]
        sh = NamedSharding(self.mesh, PartitionSpec("core"))
        self._dev = [jax.device_put(x, sh) for x in concat + zeros]

    def run(self):
        out = self.fn(*self._dev)
        self.jax.block_until_ready(out)
        return out

    def per_core(self, out_arrs):
        res = []
        for c in range(self.n_cores):
            res.append({n: np.asarray(out_arrs[i]).reshape(self.n_cores, *self.out_avals[i].shape)[c]
                        for i, n in enumerate(self.out_names)})
        return res


@functools.lru_cache(maxsize=4)
def _get_built(repeat=1, nblk_active=3):
    nc = build_module(repeat=repeat, nblk_active=nblk_active)
    return _Runner(nc, NCORES)


def kernel(**inputs):
    in_maps = _host_inputs(inputs)
    for nblk in (3, 4):
        r = _get_built(1, nblk)
        r.stage([{n: m[n] for n in r.in_names} for m in in_maps])
        outs = r.per_core(r.run())
        # safety: if any valid segment falls in a skipped block, redo with all blocks
        if nblk == 4 or not any(o["o_seg"][:, 384:].any() for o in outs):
            return _host_post(outs)
    raise RuntimeError("unreachable")
